# revision 1
# baseline (speedup 1.0000x reference)
"""Two-layer GAT (DGL GATConv style) on 8 Trainium2 NeuronCores via Bass/Tile.

Strategy (dst-partitioned graph parallel):
  - Nodes are split into 8 contiguous dst ranges; each core owns the edges
    whose dst falls in its range (host sorts/pads edge lists).
  - Per layer: each core projects its own node slice (h, el, er fall out of
    one matmul against an augmented weight matrix), packs 288B table rows
    [h bf16 x128 | el f32 x4 | er f32 x4], AllGathers the full node table,
    then per 128-edge tile gathers h_aug[src] rows with indirect DMA,
    computes edge softmax (shift-invariant, so segment-max is skipped) and
    aggregates with one bf16 matmul per tile against an on-chip-built
    one-hot segment matrix; appending the softmax weights as extra rhs
    columns yields the per-dst normalizers in the same matmul.
"""
import sys

sys.path.insert(0, "/opt/trn_rl_repo")

import math
from contextlib import ExitStack

import ml_dtypes
import numpy as np

import concourse.bass as bass
import concourse.mybir as mybir
import concourse.tile as tile
from concourse._compat import with_exitstack
from concourse.masks import make_identity

NEG = 0.2
ROW = 144  # bf16 elems per table row: h(128) + el f32(as 8) + er f32(as 8)
F = 128
H = 4
OUT = 32

bf16 = mybir.dt.bfloat16
f32 = mybir.dt.float32
i32 = mybir.dt.int32


class Cfg:
    def __init__(self, n, e, ncores, sup=32):
        self.N = n
        self.E = e
        self.NC = ncores
        self.NPC = n // ncores              # owned dsts per core
        self.NB = math.ceil(self.NPC / 128)  # dst blocks per core
        self.ROWS = self.NB * 128            # padded slice rows per core
        self.TROWS = self.ROWS * ncores
        self.SUP = sup                       # tiles per elementwise supertile
        self.TAIL = self.NPC - (self.NB - 1) * 128  # valid rows in last block
        assert self.TAIL < 128, "need at least one pad row for the sentinel"
        self.SENT = self.ROWS - 1            # sentinel table row (core 0 tail)


def prep_inputs(cfg, src, dst):
    """Host-side: per-core edge layout. Returns per-core arrays + inverse perms.

    Per core:
      offs_src [128, T] i32 : table row of edge's src (sentinel for pads)
      offs_dst [128, T] i32 : table row of edge's dst
      dloc     [128, T] bf16: dst slot within its 128-dst block
    Shared: blk_of[t], k_of[t], Tb[b] block structure (identical across cores).
    """
    src = np.asarray(src).astype(np.int64)
    dst = np.asarray(dst).astype(np.int64)
    NPC, NB, ROWS, NC = cfg.NPC, cfg.NB, cfg.ROWS, cfg.NC

    per_core = []
    perms = []
    cnts = np.zeros((NC, NB), np.int64)
    for c in range(NC):
        eid = np.nonzero(dst // NPC == c)[0]
        d0 = (dst[eid] - c * NPC).astype(np.int64)
        # degree-balance: relabel dsts so block loads are even
        counts = np.bincount(d0, minlength=NPC)
        order = np.argsort(-counts, kind="stable")
        perm = np.empty(NPC, np.int64)
        ranks = np.arange(NPC)
        perm[order] = (ranks % NB) * 128 + ranks // NB
        rd = perm[d0]
        eorder = np.argsort(rd, kind="stable")
        esrc, erd, ed0 = src[eid][eorder], rd[eorder], d0[eorder]
        bid = erd // 128
        cnts[c] = np.bincount(bid, minlength=NB)
        per_core.append((esrc, erd, ed0, bid))
        perms.append(perm)

    Tb = np.maximum(1, np.ceil(cnts.max(axis=0) / 128).astype(np.int64))
    T = int(Tb.sum())
    t0 = np.concatenate([[0], np.cumsum(Tb)])[:-1]
    blk_of = np.concatenate([np.full(Tb[b], b, np.int64) for b in range(NB)])
    k_of = np.concatenate([np.arange(Tb[b]) for b in range(NB)])

    # table-1 rows are in original local order; table-2 rows (built from the
    # layer-1 output) are in relabeled slot order — so layer 2 gets its own
    # gather index arrays mapped through the owning core's permutation.
    permcat = np.concatenate(perms)  # [NC*NPC] local slot of (core, localid)

    ins = []
    for c in range(NC):
        esrc, erd, ed0, bid = per_core[c]
        osrc = np.full((128, T), cfg.SENT, np.int32)
        odst = np.full((128, T), cfg.SENT, np.int32)
        osrc2 = np.full((128, T), cfg.SENT, np.int32)
        odst2 = np.full((128, T), cfg.SENT, np.int32)
        dl = np.zeros((128, T), np.float32)
        # edges are sorted by rd hence grouped by block
        boff = np.concatenate([[0], np.cumsum(np.bincount(bid, minlength=NB))])
        score = esrc // NPC
        srow = score * ROWS + esrc % NPC          # layer-1 src table row
        drow = c * ROWS + ed0                     # layer-1 er row (orig order)
        srow2 = score * ROWS + permcat[esrc]      # layer-2 src row (slot order)
        drow2 = c * ROWS + erd                    # layer-2 er row (slot order)
        for b in range(NB):
            j = np.arange(boff[b], boff[b + 1]) - boff[b]
            cols = t0[b] + j // 128
            parts = j % 128
            sl = slice(boff[b], boff[b + 1])
            osrc[parts, cols] = srow[sl]
            odst[parts, cols] = drow[sl]
            osrc2[parts, cols] = srow2[sl]
            odst2[parts, cols] = drow2[sl]
            dl[parts, cols] = (erd[sl] % 128).astype(np.float32)
        ins.append(
            dict(
                osrc=osrc,
                odst=odst,
                osrc2=osrc2,
                odst2=odst2,
                dloc=dl.astype(ml_dtypes.bfloat16),
            )
        )
    return ins, perms, Tb.tolist(), T, blk_of.tolist(), k_of.tolist()


def aug_weights(W, al, ar):
    """[128, 136] f32: [W | W@al_h | W@ar_h]."""
    Wa = np.zeros((F, 136), np.float32)
    Wa[:, :F] = W
    for h in range(H):
        Wa[:, F + h] = W[:, h * OUT:(h + 1) * OUT] @ al[h]
        Wa[:, F + H + h] = W[:, h * OUT:(h + 1) * OUT] @ ar[h]
    return Wa


@with_exitstack
def build_kernel(ctx: ExitStack, tc: tile.TileContext, cfg, Tb, T, blk_of, k_of,
                 dbg=False):
    nc = tc.nc
    NB, ROWS, TROWS, SUP, TAIL = cfg.NB, cfg.ROWS, cfg.TROWS, cfg.SUP, cfg.TAIL
    internal = dict(kind="ExternalOutput") if dbg else {}

    # --- I/O ---
    featT = nc.dram_tensor("featT", [F, ROWS], f32, kind="ExternalInput")
    w1 = nc.dram_tensor("w1aug", [F, 136], f32, kind="ExternalInput")
    w2 = nc.dram_tensor("w2aug", [F, 136], f32, kind="ExternalInput")
    osrc = nc.dram_tensor("osrc", [128, T], i32, kind="ExternalInput")
    odst = nc.dram_tensor("odst", [128, T], i32, kind="ExternalInput")
    osrc2 = nc.dram_tensor("osrc2", [128, T], i32, kind="ExternalInput")
    odst2 = nc.dram_tensor("odst2", [128, T], i32, kind="ExternalInput")
    dlocd = nc.dram_tensor("dloc", [128, T], bf16, kind="ExternalInput")
    sentel = nc.dram_tensor("sentel", [1, 16], bf16, kind="ExternalInput")
    iotad = nc.dram_tensor("iota", [128, 128], bf16, kind="ExternalInput")
    b1d = nc.dram_tensor("b1rep", [128, F], f32, kind="ExternalInput")
    b2d = nc.dram_tensor("b2rep", [128, OUT], f32, kind="ExternalInput")
    out_ext = nc.dram_tensor("out", [ROWS, OUT], f32, kind="ExternalOutput")

    slice1 = nc.dram_tensor("slice1", [ROWS, ROW], bf16)
    slice2 = nc.dram_tensor("slice2", [ROWS, ROW], bf16)
    table1 = nc.dram_tensor("table1", [TROWS, ROW], bf16, addr_space="Shared")
    table2 = nc.dram_tensor("table2", [TROWS, ROW], bf16, addr_space="Shared")
    h1r = nc.dram_tensor("h1r", [ROWS, F], f32, **internal)

    core_ids = list(range(cfg.NC))

    consts = ctx.enter_context(tc.tile_pool(name="consts", bufs=1))
    featp = ctx.enter_context(tc.tile_pool(name="featp", bufs=1))
    offp = ctx.enter_context(tc.tile_pool(name="offp", bufs=1))
    projp = ctx.enter_context(tc.tile_pool(name="projp", bufs=3))
    projps = ctx.enter_context(tc.tile_pool(name="projps", bufs=2, space="PSUM"))
    gp = ctx.enter_context(tc.tile_pool(name="gp", bufs=2))
    ep = ctx.enter_context(tc.tile_pool(name="ep", bufs=2))
    accp = ctx.enter_context(tc.tile_pool(name="accp", bufs=3, space="PSUM"))
    outp = ctx.enter_context(tc.tile_pool(name="outp", bufs=2))

    w1_sb = consts.tile([F, 136], f32)
    w2_sb = consts.tile([F, 136], f32)
    iota_sb = consts.tile([128, 128], bf16)
    b1_sb = consts.tile([128, F], f32)
    b2_sb = consts.tile([128, OUT], f32)
    ident = consts.tile([128, 128], f32)
    nc.sync.dma_start(w1_sb[:], w1[:])
    nc.sync.dma_start(w2_sb[:], w2[:])
    nc.sync.dma_start(iota_sb[:], iotad[:])
    nc.sync.dma_start(b1_sb[:], b1d[:])
    nc.sync.dma_start(b2_sb[:], b2d[:])
    make_identity(nc, ident[:])

    featT_sb = featp.tile([F, ROWS], f32)
    nc.sync.dma_start(featT_sb[:], featT[:])
    osrc_sb = offp.tile([128, T], i32)
    odst_sb = offp.tile([128, T], i32)
    osrc2_sb = offp.tile([128, T], i32)
    odst2_sb = offp.tile([128, T], i32)
    dloc_sb = offp.tile([128, T], bf16)
    nc.sync.dma_start(osrc_sb[:], osrc[:])
    nc.sync.dma_start(odst_sb[:], odst[:])
    nc.sync.dma_start(osrc2_sb[:], osrc2[:])
    nc.sync.dma_start(odst2_sb[:], odst2[:])
    nc.sync.dma_start(dloc_sb[:], dlocd[:])

    def emit_row_tile(ph, nt, slice_dram, sentinel_tail):
        """psum [128,136] f32 -> packed bf16 row tile -> DRAM slice.

        sentinel_tail (layer 1, original row order): rows >= TAIL of the last
        tile are sentinels (h=0, el=-1e9 so exp->0). Partition starts must be
        32-aligned, so memset an aligned superset first and overwrite the
        valid rows with the real copies.
        """
        row_t = projp.tile([128, ROW], bf16, tag="rowt")
        rv = row_t[:, :].bitcast(f32)  # [128, 72] f32 view
        if sentinel_tail and nt == NB - 1:
            astart = (TAIL // 32) * 32
            nc.vector.memset(row_t[astart:128, :], 0.0)
            nc.vector.memset(rv[astart:128, 64:68], -1e9)
            nc.vector.tensor_copy(row_t[0:TAIL, 0:F], ph[0:TAIL, 0:F])
            nc.vector.tensor_copy(rv[0:TAIL, 64:72], ph[0:TAIL, F:136])
        else:
            nc.vector.tensor_copy(row_t[:, 0:F], ph[:, 0:F])
            nc.vector.tensor_copy(rv[:, 64:72], ph[:, F:136])
        nc.sync.dma_start(slice_dram[nt * 128:(nt + 1) * 128, :], row_t[:])

    # ---------- Phase P1: project own slice with W1_aug ----------
    for nt in range(NB):
        ph = projps.tile([128, 136], f32, tag="ph")
        nc.tensor.matmul(out=ph[:], lhsT=featT_sb[:, nt * 128:(nt + 1) * 128],
                         rhs=w1_sb[:], start=True, stop=True)
        emit_row_tile(ph, nt, slice1, sentinel_tail=True)

    tc.strict_bb_all_engine_barrier()
    nc.gpsimd.collective_compute(
        "AllGather", mybir.AluOpType.bypass, replica_groups=[core_ids],
        ins=[slice1[:]], outs=[table1[:]])
    tc.strict_bb_all_engine_barrier()

    # ---------- Edge phase ----------
    def edge_phase(table, layer, os_sb, od_sb):
        acc_box = [None]
        for t0 in range(0, T, SUP):
            K = min(SUP, T - t0)
            # HW indirect DMA consumes ONE offset per partition (multi-column
            # offset APs silently gather consecutive rows) -> one call per tile.
            g = gp.tile([128, SUP * ROW], bf16, tag="g")
            erg = gp.tile([128, SUP * 8], bf16, tag="erg")
            for k in range(K):
                t = t0 + k
                nc.gpsimd.indirect_dma_start(
                    out=g[:, k * ROW:(k + 1) * ROW], out_offset=None, in_=table[:],
                    in_offset=bass.IndirectOffsetOnAxis(ap=os_sb[:, t:t + 1], axis=0))
                nc.gpsimd.indirect_dma_start(
                    out=erg[:, k * 8:(k + 1) * 8], out_offset=None, in_=table[:],
                    in_offset=bass.IndirectOffsetOnAxis(ap=od_sb[:, t:t + 1], axis=0),
                    element_offset=136)

            g32 = g[:, :].bitcast(f32)    # [128, SUP*72]
            er32 = erg[:, :K * 8].bitcast(f32)  # [128, K*4]
            logit = ep.tile([128, SUP * 4], f32, tag="logit")
            el_ap = bass.AP(tensor=g32.tensor, offset=g32.offset + 64,
                            ap=[g32.ap[0], [72, K], [1, 4]])
            nc.vector.tensor_tensor(out=logit[:, :K * 4], in0=el_ap, in1=er32,
                                    op=mybir.AluOpType.add)
            lrl = ep.tile([128, SUP * 4], f32, tag="lrl")
            nc.vector.tensor_scalar_mul(lrl[:, :K * 4], logit[:, :K * 4], NEG)
            nc.vector.tensor_tensor(out=lrl[:, :K * 4], in0=logit[:, :K * 4],
                                    in1=lrl[:, :K * 4], op=mybir.AluOpType.max)
            # clamp: sentinel logits are ~-2e8, outside the HW exp table range
            nc.vector.tensor_scalar_max(lrl[:, :K * 4], lrl[:, :K * 4], -80.0)
            p_t = ep.tile([128, SUP * 4], bf16, tag="p")
            nc.scalar.activation(p_t[:, :K * 4], lrl[:, :K * 4],
                                 mybir.ActivationFunctionType.Exp)

            s0 = ep.tile([128, SUP * 128], bf16, tag="s0")
            io = iota_sb[:, :]
            dl = dloc_sb[:, t0:t0 + K]
            iota_ap = bass.AP(tensor=io.tensor, offset=io.offset,
                              ap=[io.ap[0], [0, K], [1, 128]])
            dloc_ap = bass.AP(tensor=dl.tensor, offset=dl.offset,
                              ap=[dl.ap[0], [1, K], [0, 128]])
            nc.vector.tensor_tensor(out=s0[:, :K * 128], in0=iota_ap, in1=dloc_ap,
                                    op=mybir.AluOpType.is_equal)

            rhs = ep.tile([128, SUP * 132], bf16, tag="rhs")
            gb, pb, rb = g[:, :], p_t[:, :], rhs[:, :]
            for hh in range(H):
                in0 = bass.AP(tensor=gb.tensor, offset=gb.offset + hh * OUT,
                              ap=[gb.ap[0], [ROW, K], [1, OUT]])
                in1 = bass.AP(tensor=pb.tensor, offset=pb.offset + hh,
                              ap=[pb.ap[0], [4, K], [0, OUT]])
                o = bass.AP(tensor=rb.tensor, offset=rb.offset + hh * OUT,
                            ap=[rb.ap[0], [132, K], [1, OUT]])
                nc.vector.tensor_tensor(out=o, in0=in0, in1=in1,
                                        op=mybir.AluOpType.mult)
            pco = bass.AP(tensor=rb.tensor, offset=rb.offset + 128,
                          ap=[rb.ap[0], [132, K], [1, 4]])
            pci = bass.AP(tensor=pb.tensor, offset=pb.offset,
                          ap=[pb.ap[0], [4, K], [1, 4]])
            nc.vector.tensor_copy(out=pco, in_=pci)

            for k in range(K):
                t = t0 + k
                b = blk_of[t]
                if k_of[t] == 0:
                    acc_box[0] = accp.tile([128, 132], f32, tag="acc", name="acc")
                acc = acc_box[0]
                nc.tensor.matmul(
                    out=acc[:], lhsT=s0[:, k * 128:(k + 1) * 128],
                    rhs=rhs[:, k * 132:(k + 1) * 132],
                    start=(k_of[t] == 0), stop=(k_of[t] == Tb[b] - 1))
                if k_of[t] == Tb[b] - 1:
                    s_eps = outp.tile([128, 4], f32, tag="seps")
                    nc.vector.tensor_scalar_add(s_eps[:], acc[:, 128:132], 1e-30)
                    inv = outp.tile([128, 4], f32, tag="inv")
                    nc.vector.reciprocal(inv[:], s_eps[:])
                    iv = inv[:, :]
                    iv_ap = bass.AP(tensor=iv.tensor, offset=iv.offset,
                                    ap=[iv.ap[0], [1, 4], [0, OUT]])
                    if layer == 1:
                        of = outp.tile([128, F], f32, tag="of")
                        nc.vector.tensor_tensor(out=of[:], in0=acc[:, :F],
                                                in1=iv_ap, op=mybir.AluOpType.mult)
                        nc.vector.tensor_tensor(out=of[:], in0=of[:], in1=b1_sb[:],
                                                op=mybir.AluOpType.add)
                        nc.vector.tensor_scalar_max(of[:], of[:], 0.0)
                        nc.sync.dma_start(h1r[b * 128:(b + 1) * 128, :], of[:])
                    else:
                        nc.vector.tensor_scalar_mul(inv[:], inv[:], 0.25)
                        tmp = outp.tile([128, F], f32, tag="tmp2")
                        nc.vector.tensor_tensor(out=tmp[:], in0=acc[:, :F],
                                                in1=iv_ap, op=mybir.AluOpType.mult)
                        om = outp.tile([128, OUT], f32, tag="om")
                        tv = tmp[:, :]
                        tv_ap = bass.AP(tensor=tv.tensor, offset=tv.offset,
                                        ap=[tv.ap[0], [1, OUT], [OUT, 4]])
                        nc.vector.tensor_reduce(out=om[:], in_=tv_ap,
                                                axis=mybir.AxisListType.X,
                                                op=mybir.AluOpType.add)
                        nc.vector.tensor_tensor(out=om[:], in0=om[:], in1=b2_sb[:],
                                                op=mybir.AluOpType.add)
                        nc.sync.dma_start(out_ext[b * 128:(b + 1) * 128, :], om[:])

    edge_phase(table1, 1, osrc_sb, odst_sb)
    tc.strict_bb_all_engine_barrier()

    # ---------- Phase P2: project relu(h1) slice with W2_aug ----------
    for nt in range(NB):
        h1_t = projp.tile([128, F], f32, tag="h1t")
        nc.sync.dma_start(h1_t[:], h1r[nt * 128:(nt + 1) * 128, :])
        pt = projps.tile([128, 128], f32, tag="ptr")
        nc.tensor.transpose(out=pt[:], in_=h1_t[:], identity=ident[:])
        h1T = projp.tile([128, 128], f32, tag="h1T")
        nc.vector.tensor_copy(h1T[:], pt[:])
        ph = projps.tile([128, 136], f32, tag="ph")
        nc.tensor.matmul(out=ph[:], lhsT=h1T[:], rhs=w2_sb[:], start=True, stop=True)
        emit_row_tile(ph, nt, slice2, sentinel_tail=False)

    # table-2 is in relabeled slot order; the sentinel slot (last row) must
    # still read as "no edge": patch its el to -1e9 (DRAM->DRAM DMA crashes
    # neuronxcc, so bounce through SBUF).
    sent_sb = consts.tile([1, 16], bf16)
    nc.sync.dma_start(sent_sb[:], sentel[:])
    tc.strict_bb_all_engine_barrier()
    nc.sync.dma_start(slice2[ROWS - 1:ROWS, 128:144], sent_sb[:])
    tc.strict_bb_all_engine_barrier()
    nc.gpsimd.collective_compute(
        "AllGather", mybir.AluOpType.bypass, replica_groups=[core_ids],
        ins=[slice2[:]], outs=[table2[:]])
    tc.strict_bb_all_engine_barrier()

    edge_phase(table2, 2, osrc2_sb, odst2_sb)

    if dbg:
        dbg1 = nc.dram_tensor("dbg1", [ROWS, ROW], bf16, kind="ExternalOutput")
        dbg2 = nc.dram_tensor("dbg2", [ROWS, ROW], bf16, kind="ExternalOutput")
        tc.strict_bb_all_engine_barrier()
        for nt in range(NB):
            for srcd, dstd in ((slice1, dbg1), (slice2, dbg2)):
                bt = projp.tile([128, ROW], bf16, tag="dbgb", name="dbgb")
                nc.sync.dma_start(bt[:], srcd[nt * 128:(nt + 1) * 128, :])
                nc.sync.dma_start(dstd[nt * 128:(nt + 1) * 128, :], bt[:])


def build_nc(cfg, Tb, T, blk_of, k_of, compile=True, dbg=False):
    from concourse import bacc

    nc = bacc.Bacc("TRN2", target_bir_lowering=False)
    with tile.TileContext(nc) as tc:
        build_kernel(tc, cfg, Tb, T, blk_of, k_of, dbg=dbg)
    if compile:
        nc.compile()
    return nc


def make_in_maps(cfg, per_core_edges, feat, W1, al1, ar1, b1, W2, al2, ar2, b2):
    w1a = aug_weights(np.asarray(W1, np.float32), np.asarray(al1, np.float32),
                      np.asarray(ar1, np.float32))
    w2a = aug_weights(np.asarray(W2, np.float32), np.asarray(al2, np.float32),
                      np.asarray(ar2, np.float32))
    iota = np.broadcast_to(np.arange(128, dtype=np.float32), (128, 128))
    iota = np.ascontiguousarray(iota.astype(ml_dtypes.bfloat16))
    sentel = np.full((1, 8), -1e9, np.float32).view(np.uint16).reshape(1, 16)
    sentel = sentel.view(ml_dtypes.bfloat16)
    b1r = np.ascontiguousarray(np.broadcast_to(
        np.asarray(b1, np.float32).reshape(1, F), (128, F)))
    b2m = np.asarray(b2, np.float32).reshape(H, OUT).mean(axis=0)
    b2r = np.ascontiguousarray(np.broadcast_to(b2m.reshape(1, OUT), (128, OUT)))
    feat = np.asarray(feat, np.float32)
    in_maps = []
    for c in range(cfg.NC):
        fslice = np.zeros((F, cfg.ROWS), np.float32)
        fslice[:, :cfg.NPC] = feat[c * cfg.NPC:(c + 1) * cfg.NPC].T
        m = dict(
            featT=fslice,
            w1aug=w1a, w2aug=w2a,
            osrc=per_core_edges[c]["osrc"],
            odst=per_core_edges[c]["odst"],
            osrc2=per_core_edges[c]["osrc2"],
            odst2=per_core_edges[c]["odst2"],
            dloc=per_core_edges[c]["dloc"],
            iota=iota, b1rep=b1r, b2rep=b2r, sentel=sentel,
        )
        in_maps.append(m)
    return in_maps


_CACHE = {}


def _get_program(cfg, src, dst):
    per_core, perms, Tb, T, blk_of, k_of = prep_inputs(cfg, src, dst)
    key = (cfg.N, cfg.E, cfg.NC, tuple(Tb), tuple(blk_of), tuple(k_of))
    if key not in _CACHE:
        _CACHE[key] = build_nc(cfg, Tb, T, blk_of, k_of)
    return _CACHE[key], per_core, perms


def kernel(feat, src, dst, W1, al1, ar1, b1, W2, al2, ar2, b2,
           _trace=False, _return_results=False):
    from concourse.bass_utils import run_bass_kernel_spmd

    cfg = Cfg(100000, 800000, 8)
    nc, per_core, perms = _get_program(cfg, src, dst)
    in_maps = make_in_maps(cfg, per_core, feat, W1, al1, ar1, b1,
                           W2, al2, ar2, b2)
    res = run_bass_kernel_spmd(nc, in_maps, list(range(cfg.NC)), trace=_trace)
    out = np.zeros((cfg.N, OUT), np.float32)
    for c in range(cfg.NC):
        oc = np.asarray(res.results[c]["out"])  # [ROWS, 32], rows are relabeled
        out[c * cfg.NPC:(c + 1) * cfg.NPC] = oc[perms[c]]
    if _return_results:
        return out, res
    return out



# revision 5
# speedup vs baseline: 1.8571x; 1.8571x over previous
"""Two-layer GAT (DGL GATConv) on 8 Trainium2 NeuronCores via Bass/Tile.

v2: dst-partitioned graph parallel with dma_gather edge gathers.

  - Nodes are slot-relabeled per core (greedy block assignment balancing
    per-(block, src-window) edge counts); everything on device is in slot
    order, so both layers share one gather-index array and one one-hot
    structure.
  - Per layer: project own slice (h, el, er from one matmul vs an augmented
    weight matrix), pack 512B table rows [h0|1|h1|1|h2|1|h3|1 bf16 | el f32],
    AllGather the table, then gather per-edge src rows with int16 dma_gather
    (4 windows of 2 core-slices each keep indices < 32768), compute edge
    softmax and aggregate per 128-dst block with one bf16 matmul per tile.
    The interleaved "1" columns make the same matmul emit the per-dst softmax
    normalizers. er[dst] is produced on-chip per tile by transposing the
    one-hot on the PE array and multiplying with the SBUF-resident er table.
"""
import sys

sys.path.insert(0, "/opt/trn_rl_repo")

import math
from contextlib import ExitStack

import ml_dtypes
import numpy as np

import concourse.bass as bass
import concourse.mybir as mybir
import concourse.tile as tile
from concourse._compat import with_exitstack
from concourse.masks import make_identity

NEG = 0.2
F = 128
H = 4
OUT = 32
ROWB = 256          # bf16 elems per table row (512 B)
NC = 8
NPC = 12500
NB = 98             # 128-dst blocks per core
ROWS = NB * 128     # 12544 slots per core
SENT = ROWS - 1     # reserved pad slot on every core (block 97 capped at 127)
TROWS = ROWS * NC
WIN = 4
WROWS = 2 * ROWS    # rows per gather window (pair of core slices), < 32768
GROUP = 4           # dst blocks per PSUM accumulation group
SENT_EL = -80.0     # sentinel el -> exp(lrelu(-80+er)) ~ 1e-7

bf16 = mybir.dt.bfloat16
f32 = mybir.dt.float32
i16 = mybir.dt.int16


# ---------------------------------------------------------------- host prep

def _shared_structure(n3):
    """Per-(block, window) tile budgets + global tile ordering."""
    tb = np.full((NB, WIN), 2, np.int64)
    for w in range(WIN):
        big = (np.arange(n3) * NB // n3 + w * 7) % NB
        tb[np.unique(big), w] = 3
    groups = [list(range(g * GROUP, min(NB, (g + 1) * GROUP)))
              for g in range(math.ceil(NB / GROUP))]
    runs = []           # dict(w, t0, tiles=[(b, start, stop)], fin=[blocks])
    tiles_bw = [[None] * WIN for _ in range(NB)]
    t = 0
    for blocks in groups:
        for w in range(WIN):
            tl = []
            for b in blocks:
                tiles_bw[b][w] = np.arange(t + len(tl), t + len(tl) + tb[b, w])
                for k in range(tb[b, w]):
                    tl.append((b, w == 0 and k == 0,
                               w == WIN - 1 and k == tb[b, WIN - 1] - 1))
            runs.append(dict(w=w, t0=t, tiles=tl,
                             fin=blocks if w == WIN - 1 else []))
            t += len(tl)
    return tb, runs, tiles_bw, t


def _assign_blocks(wvec, tb):
    """Greedy: assign dsts (with per-window edge counts) to blocks under
    per-(b,w) capacity tb*128 and per-block dst capacity."""
    cap = tb * 128
    capd = np.full(NB, 128, np.int64)
    capd[NB - 1] = 127          # reserve SENT slot
    deg = wvec.sum(1)
    order = np.argsort(-deg, kind="stable")
    cnt = np.zeros((NB, WIN), np.int64)
    ndst = np.zeros(NB, np.int64)
    blk = np.empty(NPC, np.int64)
    slot_in = np.empty(NPC, np.int64)
    for d in order:
        resid = cap - cnt - wvec[d]
        ok = (resid.min(1) >= 0) & (ndst < capd)
        if not ok.any():
            return None, None
        score = np.where(ok, resid.min(1) * 1000 - ndst, -(10 ** 9))
        b = int(np.argmax(score))
        blk[d] = b
        slot_in[d] = ndst[b]
        cnt[b] += wvec[d]
        ndst[b] += 1
    return blk * 128 + slot_in, cnt


def prep_inputs(src, dst):
    src = np.asarray(src).astype(np.int64)
    dst = np.asarray(dst).astype(np.int64)
    win_edge = src // (2 * NPC)          # gather window of each edge (by src)

    n3 = 8
    while True:
        tb, runs, tiles_bw, T = _shared_structure(n3)
        perms = []
        ecore = []
        ok = True
        for c in range(NC):
            eid = np.nonzero((dst >= c * NPC) & (dst < (c + 1) * NPC))[0]
            d0 = dst[eid] - c * NPC
            wv = win_edge[eid]
            wvec = np.zeros((NPC, WIN), np.int64)
            np.add.at(wvec, (d0, wv), 1)
            perm, _ = _assign_blocks(wvec, tb)
            if perm is None:
                ok = False
                break
            perms.append(perm)
            ecore.append((eid, d0, wv))
        if ok:
            break
        n3 += 4
        assert n3 <= 32, "edge packing infeasible"

    rowof = np.empty(src.max() + 1 if False else NC * NPC, np.int64)
    for c in range(NC):
        rowof[c * NPC:(c + 1) * NPC] = c * ROWS + perms[c]

    per_core = []
    for c in range(NC):
        eid, d0, wv = ecore[c]
        perm = perms[c]
        osflat = np.full(T * 128, SENT, np.int32)
        dlflat = np.full(T * 128, 127, np.float32)
        b_e = perm[d0] // 128
        slot_e = (perm[d0] % 128).astype(np.float32)
        relrow = (rowof[src[eid]] - wv * WROWS).astype(np.int32)
        key = b_e * WIN + wv
        order_e = np.argsort(key, kind="stable")
        counts = np.bincount(key, minlength=NB * WIN)
        off = np.concatenate([[0], np.cumsum(counts)])
        for b in range(NB):
            for w in range(WIN):
                j0, j1 = off[b * WIN + w], off[b * WIN + w + 1]
                if j1 == j0:
                    continue
                es = order_e[j0:j1]
                jj = np.arange(j1 - j0)
                pos = tiles_bw[b][w][jj // 128] * 128 + jj % 128
                osflat[pos] = relrow[es]
                dlflat[pos] = slot_e[es]
        assert osflat.max() < WROWS and osflat.min() >= 0
        osw = np.tile(
            np.ascontiguousarray(
                osflat.astype(np.int16).reshape(T * 8, 16).T), (8, 1))
        dl = np.ascontiguousarray(
            dlflat.reshape(T, 128).T).astype(ml_dtypes.bfloat16)
        per_core.append(dict(osw=osw, dloc=dl))
    return per_core, perms, tb, runs, T


def aug_weights(W, al, ar):
    """[128, 136] f32: [W | W@al_h | W@ar_h]."""
    Wa = np.zeros((F, 136), np.float32)
    Wa[:, :F] = W
    for h in range(H):
        Wa[:, F + h] = W[:, h * OUT:(h + 1) * OUT] @ al[h]
        Wa[:, F + H + h] = W[:, h * OUT:(h + 1) * OUT] @ ar[h]
    return Wa


# ---------------------------------------------------------------- kernel

@with_exitstack
def build_kernel(ctx: ExitStack, tc: tile.TileContext, runs, T, supmax):
    nc = tc.nc

    featT = nc.dram_tensor("featT", [F, ROWS], f32, kind="ExternalInput")
    w1 = nc.dram_tensor("w1aug", [F, 136], f32, kind="ExternalInput")
    w2 = nc.dram_tensor("w2aug", [F, 136], bf16, kind="ExternalInput")
    osw_d = nc.dram_tensor("osw", [128, T * 8], i16, kind="ExternalInput")
    dloc_d = nc.dram_tensor("dloc", [128, T], bf16, kind="ExternalInput")
    iota_d = nc.dram_tensor("iota", [128, 128], bf16, kind="ExternalInput")
    b1_d = nc.dram_tensor("b1rep", [128, F], f32, kind="ExternalInput")
    b2_d = nc.dram_tensor("b2rep", [128, OUT], f32, kind="ExternalInput")
    sent_d = nc.dram_tensor("sentel", [1, 8], bf16, kind="ExternalInput")
    out_ext = nc.dram_tensor("out", [ROWS, OUT], f32, kind="ExternalOutput")

    slice1 = nc.dram_tensor("slice1", [ROWS, ROWB], bf16)
    slice2 = nc.dram_tensor("slice2", [ROWS, ROWB], bf16)
    table1 = nc.dram_tensor("table1", [TROWS, ROWB], bf16, addr_space="Shared")
    table2 = nc.dram_tensor("table2", [TROWS, ROWB], bf16, addr_space="Shared")
    h1r = nc.dram_tensor("h1r", [ROWS, F], bf16)

    core_ids = list(range(NC))

    consts = ctx.enter_context(tc.tile_pool(name="consts", bufs=1))
    offp = ctx.enter_context(tc.tile_pool(name="offp", bufs=1))
    erp = ctx.enter_context(tc.tile_pool(name="erp", bufs=1))

    w1_sb = consts.tile([F, 136], f32)
    w2_sb = consts.tile([F, 136], bf16)
    iota_sb = consts.tile([128, 128], bf16)
    b1_sb = consts.tile([128, F], f32)
    b2_sb = consts.tile([128, OUT], f32)
    ident = consts.tile([128, 128], bf16)
    sent_sb = consts.tile([1, 8], bf16)
    nc.sync.dma_start(w1_sb[:], w1[:])
    nc.sync.dma_start(w2_sb[:], w2[:])
    nc.sync.dma_start(iota_sb[:], iota_d[:])
    nc.sync.dma_start(b1_sb[:], b1_d[:])
    nc.sync.dma_start(b2_sb[:], b2_d[:])
    nc.sync.dma_start(sent_sb[:], sent_d[:])
    make_identity(nc, ident[:])

    osw_sb = offp.tile([128, T * 8], i16)
    dloc_sb = offp.tile([128, T], bf16)
    nc.sync.dma_start(osw_sb[:], osw_d[:])
    nc.sync.dma_start(dloc_sb[:], dloc_d[:])

    er1_sb = erp.tile([128, NB * 4], bf16)
    er2_sb = erp.tile([128, NB * 4], bf16)

    def proj_phase(get_lhsT, w_sb, slice_d, er_sb):
        with tc.tile_pool(name="php", bufs=2, space="PSUM") as php, \
             tc.tile_pool(name="projp", bufs=2) as projp:
            for nt in range(NB):
                ph = php.tile([128, 136], f32, tag="ph")
                nc.tensor.matmul(out=ph[:], lhsT=get_lhsT(nt), rhs=w_sb[:],
                                 start=True, stop=True)
                row_t = projp.tile([128, ROWB], bf16, tag="rowt")
                rv = row_t[:, :].bitcast(f32)
                ro = row_t[:, :]
                # h heads -> interleaved 33-blocks [h(32) | 1]
                o_ap = bass.AP(tensor=ro.tensor, offset=ro.offset,
                               ap=[ro.ap[0], [33, 4], [1, 32]])
                pv = ph[:, :]
                i_ap = bass.AP(tensor=pv.tensor, offset=pv.offset,
                               ap=[pv.ap[0], [32, 4], [1, 32]])
                nc.vector.tensor_copy(out=o_ap, in_=i_ap)
                ones_ap = bass.AP(tensor=ro.tensor, offset=ro.offset + 32,
                                  ap=[ro.ap[0], [33, 4], [1, 1]])
                nc.vector.memset(ones_ap, 1.0)
                nc.vector.memset(row_t[:, 132:136], 0.0)
                nc.vector.memset(row_t[:, 144:ROWB], 0.0)
                nc.vector.tensor_copy(out=rv[:, 68:72], in_=ph[:, 128:132])
                nc.vector.tensor_copy(out=er_sb[:, nt * 4:(nt + 1) * 4],
                                      in_=ph[:, 132:136])
                nc.sync.dma_start(slice_d[nt * 128:(nt + 1) * 128, :], row_t[:])
            tc.strict_bb_all_engine_barrier()
            # sentinel slot: el := -80 (bf16 cols 136:144 = f32 el field)
            nc.sync.dma_start(slice_d[SENT:SENT + 1, 136:144], sent_sb[:])

    def edge_phase(table, er_sb, layer):
        with tc.tile_pool(name="accp", bufs=GROUP, space="PSUM") as accp, \
             tc.tile_pool(name="tpp", bufs=2, space="PSUM") as tpp, \
             tc.tile_pool(name="erps", bufs=2, space="PSUM") as erps, \
             tc.tile_pool(name="gp", bufs=2) as gp, \
             tc.tile_pool(name="ep", bufs=2) as ep, \
             tc.tile_pool(name="outp", bufs=2) as outp:
            acc_t = {}
            for run in runs:
                w, t0, tiles = run["w"], run["t0"], run["tiles"]
                n = len(tiles)
                g_t = gp.tile([128, supmax, ROWB], bf16, tag="g")
                # Q7 dma_gather caps at 1024 indices per call
                for c0 in range(0, n, 8):
                    cn = min(8, n - c0)
                    nc.gpsimd.dma_gather(
                        g_t[:, c0:c0 + cn, :],
                        table[w * WROWS:(w + 1) * WROWS, :],
                        osw_sb[:, (t0 + c0) * 8:(t0 + c0 + cn) * 8],
                        cn * 128, cn * 128, ROWB)

                s0_t = ep.tile([128, supmax * 128], bf16, tag="s0")
                io = iota_sb[:, :]
                dl = dloc_sb[:, t0:t0 + n]
                iota_ap = bass.AP(tensor=io.tensor, offset=io.offset,
                                  ap=[io.ap[0], [0, n], [1, 128]])
                dloc_ap = bass.AP(tensor=dl.tensor, offset=dl.offset,
                                  ap=[dl.ap[0], [1, n], [0, 128]])
                nc.vector.tensor_tensor(out=s0_t[:, :n * 128], in0=iota_ap,
                                        in1=dloc_ap,
                                        op=mybir.AluOpType.is_equal)

                er_ps = erps.tile([128, supmax * 4], f32, tag="erps")
                for k, (b, st, sp) in enumerate(tiles):
                    s0T_ps = tpp.tile([128, 128], bf16, tag="s0T")
                    nc.tensor.transpose(s0T_ps[:],
                                        s0_t[:, k * 128:(k + 1) * 128],
                                        ident[:])
                    s0T_sb = ep.tile([128, 128], bf16, tag="s0Ts")
                    if k % 2 == 0:
                        nc.scalar.copy(s0T_sb[:], s0T_ps[:])
                    else:
                        nc.vector.tensor_copy(s0T_sb[:], s0T_ps[:])
                    nc.tensor.matmul(out=er_ps[:, k * 4:(k + 1) * 4],
                                     lhsT=s0T_sb[:],
                                     rhs=er_sb[:, b * 4:(b + 1) * 4],
                                     start=True, stop=True,
                                     skip_group_check=True)

                g32 = g_t[:, :, :].bitcast(f32)
                el_ap = bass.AP(tensor=g32.tensor, offset=g32.offset + 68,
                                ap=[g32.ap[0], [128, n], [1, 4]])
                lg = ep.tile([128, supmax * 4], f32, tag="lg")
                nc.vector.tensor_tensor(out=lg[:, :n * 4], in0=el_ap,
                                        in1=er_ps[:, :n * 4],
                                        op=mybir.AluOpType.add)
                lrl = ep.tile([128, supmax * 4], f32, tag="lrl")
                nc.vector.tensor_scalar_mul(lrl[:, :n * 4], lg[:, :n * 4], NEG)
                nc.vector.tensor_tensor(out=lrl[:, :n * 4], in0=lg[:, :n * 4],
                                        in1=lrl[:, :n * 4],
                                        op=mybir.AluOpType.max)
                p_t = ep.tile([128, supmax * 4], bf16, tag="p")
                nc.scalar.activation(p_t[:, :n * 4], lrl[:, :n * 4],
                                     mybir.ActivationFunctionType.Exp)

                rhs_t = ep.tile([128, supmax * 132], bf16, tag="rhs")
                gb, pb, rb = g_t[:, :, :], p_t[:, :], rhs_t[:, :]
                in0 = bass.AP(tensor=gb.tensor, offset=gb.offset,
                              ap=[gb.ap[0], [ROWB, n], [33, 4], [1, 33]])
                in1 = bass.AP(tensor=pb.tensor, offset=pb.offset,
                              ap=[pb.ap[0], [4, n], [1, 4], [0, 33]])
                o = bass.AP(tensor=rb.tensor, offset=rb.offset,
                            ap=[rb.ap[0], [132, n], [33, 4], [1, 33]])
                nc.vector.tensor_tensor(out=o, in0=in0, in1=in1,
                                        op=mybir.AluOpType.mult)

                for k, (b, st, sp) in enumerate(tiles):
                    if st:
                        acc_t[b] = accp.tile([128, 132], f32, tag="acc",
                                             name=f"acc_l{layer}_b{b}")
                    nc.tensor.matmul(out=acc_t[b][:],
                                     lhsT=s0_t[:, k * 128:(k + 1) * 128],
                                     rhs=rhs_t[:, k * 132:(k + 1) * 132],
                                     start=st, stop=sp, skip_group_check=True)

                if run["fin"]:
                    finalize(run["fin"], acc_t, layer, outp)

    def finalize(blocks, acc_t, layer, outp):
        nb = len(blocks)
        stag = outp.tile([128, GROUP * 132], f32, tag="stag")
        for j, b in enumerate(blocks):
            nc.scalar.copy(stag[:, j * 132:(j + 1) * 132], acc_t[b][:])
        sv = stag[:, :]
        s_ap = bass.AP(tensor=sv.tensor, offset=sv.offset + 32,
                       ap=[sv.ap[0], [132, nb], [33, 4]])
        seps = outp.tile([128, GROUP * 4], f32, tag="seps")
        nc.vector.tensor_scalar_add(seps[:, :nb * 4], s_ap, 1e-30)
        inv = outp.tile([128, GROUP * 4], f32, tag="inv")
        nc.vector.reciprocal(inv[:, :nb * 4], seps[:, :nb * 4])
        if layer == 2:
            nc.vector.tensor_scalar_mul(inv[:, :nb * 4], inv[:, :nb * 4], 0.25)
        of = outp.tile([128, GROUP * 128], f32, tag="of")
        ov, iv = of[:, :], inv[:, :]
        msg_ap = bass.AP(tensor=sv.tensor, offset=sv.offset,
                         ap=[sv.ap[0], [132, nb], [33, 4], [1, 32]])
        inv_ap = bass.AP(tensor=iv.tensor, offset=iv.offset,
                         ap=[iv.ap[0], [4, nb], [1, 4], [0, 32]])
        of_ap = bass.AP(tensor=ov.tensor, offset=ov.offset,
                        ap=[ov.ap[0], [128, nb], [32, 4], [1, 32]])
        nc.vector.tensor_tensor(out=of_ap, in0=msg_ap, in1=inv_ap,
                                op=mybir.AluOpType.mult)
        if layer == 1:
            bv = b1_sb[:, :]
            b1_ap = bass.AP(tensor=bv.tensor, offset=bv.offset,
                            ap=[bv.ap[0], [0, nb], [1, F]])
            of2_ap = bass.AP(tensor=ov.tensor, offset=ov.offset,
                             ap=[ov.ap[0], [128, nb], [1, F]])
            nc.vector.tensor_tensor(out=of2_ap, in0=of2_ap, in1=b1_ap,
                                    op=mybir.AluOpType.add)
            h1row = outp.tile([128, GROUP * 128], bf16, tag="h1row")
            nc.scalar.activation(h1row[:, :nb * 128], of[:, :nb * 128],
                                 mybir.ActivationFunctionType.Relu)
            for j, b in enumerate(blocks):
                nc.sync.dma_start(h1r[b * 128:(b + 1) * 128, :],
                                  h1row[:, j * 128:(j + 1) * 128])
        else:
            rd = outp.tile([128, GROUP * OUT], f32, tag="rd")
            red_in = bass.AP(tensor=ov.tensor, offset=ov.offset,
                             ap=[ov.ap[0], [128, nb], [1, 32], [32, 4]])
            rv2 = rd[:, :]
            red_out = bass.AP(tensor=rv2.tensor, offset=rv2.offset,
                              ap=[rv2.ap[0], [32, nb], [1, 32]])
            nc.vector.tensor_reduce(out=red_out, in_=red_in,
                                    axis=mybir.AxisListType.X,
                                    op=mybir.AluOpType.add)
            bv2 = b2_sb[:, :]
            b2_ap = bass.AP(tensor=bv2.tensor, offset=bv2.offset,
                            ap=[bv2.ap[0], [0, nb], [1, OUT]])
            nc.vector.tensor_tensor(out=red_out, in0=red_out, in1=b2_ap,
                                    op=mybir.AluOpType.add)
            for j, b in enumerate(blocks):
                nc.sync.dma_start(out_ext[b * 128:(b + 1) * 128, :],
                                  rd[:, j * OUT:(j + 1) * OUT])

    # ---------------- phase sequence ----------------
    with tc.tile_pool(name="featp", bufs=1) as featp:
        featT_sb = featp.tile([F, ROWS], f32)
        nc.sync.dma_start(featT_sb[:], featT[:])
        proj_phase(lambda nt: featT_sb[:, nt * 128:(nt + 1) * 128],
                   w1_sb, slice1, er1_sb)
        tc.strict_bb_all_engine_barrier()
        nc.gpsimd.collective_compute(
            "AllGather", mybir.AluOpType.bypass, replica_groups=[core_ids],
            ins=[slice1[:]], outs=[table1[:]])
        tc.strict_bb_all_engine_barrier()
        edge_phase(table1, er1_sb, 1)

    tc.strict_bb_all_engine_barrier()
    with tc.tile_pool(name="h1p", bufs=1) as h1p:
        h1T_sb = h1p.tile([128, ROWS], bf16)
        nc.sync.dma_start(h1T_sb[:], h1r[:], transpose=True)
        proj_phase(lambda nt: h1T_sb[:, nt * 128:(nt + 1) * 128],
                   w2_sb, slice2, er2_sb)
        tc.strict_bb_all_engine_barrier()
        nc.gpsimd.collective_compute(
            "AllGather", mybir.AluOpType.bypass, replica_groups=[core_ids],
            ins=[slice2[:]], outs=[table2[:]])
        tc.strict_bb_all_engine_barrier()
        edge_phase(table2, er2_sb, 2)


def build_nc(runs, T, compile=True):
    from concourse import bacc

    supmax = max(len(r["tiles"]) for r in runs)
    nc = bacc.Bacc("TRN2", target_bir_lowering=False)
    with tile.TileContext(nc) as tc:
        build_kernel(tc, runs, T, supmax)
    if compile:
        nc.compile()
    return nc


def make_in_maps(per_core, feat, perms, W1, al1, ar1, b1, W2, al2, ar2, b2):
    w1a = aug_weights(np.asarray(W1, np.float32), np.asarray(al1, np.float32),
                      np.asarray(ar1, np.float32))
    w2a = aug_weights(np.asarray(W2, np.float32), np.asarray(al2, np.float32),
                      np.asarray(ar2, np.float32)).astype(ml_dtypes.bfloat16)
    iota = np.broadcast_to(np.arange(128, dtype=np.float32), (128, 128))
    iota = np.ascontiguousarray(iota.astype(ml_dtypes.bfloat16))
    sentel = np.full((1, 4), SENT_EL, np.float32).view(np.uint16).reshape(1, 8)
    sentel = sentel.view(ml_dtypes.bfloat16)
    b1r = np.ascontiguousarray(np.broadcast_to(
        np.asarray(b1, np.float32).reshape(1, F), (128, F)))
    b2m = np.asarray(b2, np.float32).reshape(H, OUT).mean(axis=0)
    b2r = np.ascontiguousarray(np.broadcast_to(b2m.reshape(1, OUT), (128, OUT)))
    feat = np.asarray(feat, np.float32)
    in_maps = []
    for c in range(NC):
        fs = np.zeros((ROWS, F), np.float32)
        fs[perms[c]] = feat[c * NPC:(c + 1) * NPC]
        m = dict(
            featT=np.ascontiguousarray(fs.T),
            w1aug=w1a, w2aug=w2a,
            osw=per_core[c]["osw"],
            dloc=per_core[c]["dloc"],
            iota=iota, b1rep=b1r, b2rep=b2r, sentel=sentel,
        )
        in_maps.append(m)
    return in_maps


_CACHE = {}


def _get_program(src, dst):
    per_core, perms, tb, runs, T = prep_inputs(src, dst)
    key = (T, tb.tobytes())
    if key not in _CACHE:
        _CACHE[key] = build_nc(runs, T)
    return _CACHE[key], per_core, perms


def kernel(feat, src, dst, W1, al1, ar1, b1, W2, al2, ar2, b2,
           _trace=False, _return_results=False):
    from concourse.bass_utils import run_bass_kernel_spmd

    nc, per_core, perms = _get_program(src, dst)
    in_maps = make_in_maps(per_core, feat, perms, W1, al1, ar1, b1,
                           W2, al2, ar2, b2)
    res = run_bass_kernel_spmd(nc, in_maps, list(range(NC)), trace=_trace)
    out = np.zeros((NC * NPC, OUT), np.float32)
    for c in range(NC):
        oc = np.asarray(res.results[c]["out"])
        out[c * NPC:(c + 1) * NPC] = oc[perms[c]]
    if _return_results:
        return out, res
    return out
